# revision 1
# baseline (speedup 1.0000x reference)
"""Trainium2 Bass kernel for nn_ContentExtracctor (retrieval_knn).

out[0, :, t] = proj_w @ mean_j lut[0, :, idx_j(t)] + proj_b
where idx(t) = top-4 indices of cosine similarity between x[0,:,t] and
lut columns.

Sharding: T=8192 split across 8 cores (1024 queries each), lut replicated.

Per-core algorithm (all scoring in exact fp32):
  - norms2[n] = sum_d lut[d,n]^2 via ACT Square + ones@lsq matmul (fp32)
  - rnorm = rsqrt(norms2) via ACT sqrt + DVE reciprocal + 1 Newton step
  - lut_hat = lut * rnorm (column-normalized; query norm doesn't change
    per-row top-k ordering so x is left unnormalized)
  - G = x^T @ lut_hat (fp32 matmul), streamed over 8 column-octants
  - per octant: top-8 values+indices per query row (DVE max8/max_index)
  - merge 64 candidates/query -> top-4 indices (exact fp32 scores)
  - P^T[n, :] = 0.25*(proj_w @ lut[:,n] + proj_b) stored to DRAM;
    gather 4 rows per query (indirect DMA), sum -> output
"""
import numpy as np

import concourse.bass as bass
import concourse.bacc as bacc
import concourse.mybir as mybir
import concourse.tile as tile
from concourse import bass_utils
from concourse.masks import make_identity

P = 128
B = 1
D = 768
T = 8192
N = 16384
C = 96
K = 4
NCORES = 8
TSH = T // NCORES         # 1024 queries per core
NT = TSH // P             # 8 query tiles per core
NCH = D // P              # 6 contraction chunks
NO = 16                   # column blocks
NOCT = N // NO            # 1024 columns per block
NB = NOCT // 512          # 4 psum chunks per octant

f32 = mybir.dt.float32
u32 = mybir.dt.uint32
i32 = mybir.dt.int32
AF = mybir.ActivationFunctionType


def build_kernel():
    nc = bacc.Bacc("TRN2", target_bir_lowering=False, debug=False)

    xs_d = nc.dram_tensor("xs", [D, TSH], f32, kind="ExternalInput")
    lut_d = nc.dram_tensor("lut", [D, N], f32, kind="ExternalInput")
    pw_d = nc.dram_tensor("projw", [C, D], f32, kind="ExternalInput")
    pb_d = nc.dram_tensor("projb", [C, 1], f32, kind="ExternalInput")
    out_d = nc.dram_tensor("out", [C, TSH], f32, kind="ExternalOutput")
    pt_d = nc.dram_tensor("pt", [N, C], f32, kind="Internal")

    with tile.TileContext(nc) as tc:
        with (
            tc.tile_pool(name="cst", bufs=1) as cst,
            tc.tile_pool(name="sb", bufs=2) as sb,
            tc.tile_pool(name="gp", bufs=3) as gp,
            tc.tile_pool(name="ps", bufs=2, space="PSUM") as ps,
            tc.tile_pool(name="psn", bufs=1, space="PSUM") as psn,
        ):
            # ---- constants / setup ----
            x_all = cst.tile([P, NCH * TSH], f32, name="x_all")
            nc.sync.dma_start(
                out=x_all[:].rearrange("p (c t) -> p c t", c=NCH),
                in_=xs_d.rearrange("(c p) t -> p c t", p=P))

            pw_sb = cst.tile([C, D], f32, name="pw_sb")
            nc.sync.dma_start(out=pw_sb[:], in_=pw_d[:, :])
            # fold the 1/k mean into proj weights and bias
            nc.vector.tensor_scalar_mul(pw_sb[:], pw_sb[:], 1.0 / K)
            pb_sb = cst.tile([C, 1], f32, name="pb_sb")
            nc.sync.dma_start(out=pb_sb[:], in_=pb_d[:, :])

            ident = cst.tile([P, P], f32, name="ident")
            make_identity(nc, ident[:])

            # projT [128, NCH*C]: chunk c holds proj_w[:, c*128:(c+1)*128]^T
            projT = cst.tile([P, NCH * C], f32, name="projT")
            for ci in range(NCH):
                tps = ps.tile([P, C], f32, name="tps", tag="tps", bufs=1)
                nc.tensor.transpose(
                    out=tps[:], in_=pw_sb[:, ci * P:(ci + 1) * P],
                    identity=ident[0:C, 0:C])
                nc.vector.tensor_copy(out=projT[:, ci * C:(ci + 1) * C],
                                      in_=tps[:])

            ones = cst.tile([P, P], f32, name="ones")
            nc.vector.memset(ones[:], 1.0)

            iota64 = cst.tile([P, NO * 8], i32, name="iota64")
            nc.gpsimd.iota(iota64[:], pattern=[[1, NO * 8]], base=0,
                           channel_multiplier=0)
            iota64f = cst.tile([P, NO * 8], f32, name="iota64f")
            nc.vector.tensor_copy(out=iota64f[:], in_=iota64[:])

            # candidate arrays per query tile (values + global indices, f32)
            cvals = [cst.tile([P, NO * 8], f32, name=f"cvals{t}")
                     for t in range(NT)]
            cidxf = [cst.tile([P, NO * 8], f32, name=f"cidxf{t}")
                     for t in range(NT)]

            # ---- octant loop ----
            for o in range(NO):
                n0 = o * NOCT
                lut_o = sb.tile([P, NCH * NOCT], f32, name="lut_o", tag="lut")
                nc.sync.dma_start(
                    out=lut_o[:].rearrange("p (c n) -> p c n", c=NCH),
                    in_=lut_d[:, n0:n0 + NOCT].rearrange(
                        "(c p) n -> p c n", p=P))

                def lch(c):
                    return lut_o[:, c * NOCT:(c + 1) * NOCT]

                # squared-column-sums -> psum_n [128, NOCT] (replicated rows)
                psum_n = psn.tile([P, NOCT], f32, name="psum_n", tag="pn")
                for ci in range(NCH):
                    lsq = sb.tile([P, NOCT], f32, name="lsq", tag="lsq")
                    nc.scalar.activation(lsq[:], lch(ci), AF.Square)
                    for b in range(NB):
                        nc.tensor.matmul(
                            out=psum_n[:, b * 512:(b + 1) * 512],
                            lhsT=ones[:],
                            rhs=lsq[:, b * 512:(b + 1) * 512],
                            start=(ci == 0), stop=(ci == NCH - 1))

                # P matmul on raw lut (fp32) -> P^T rows to DRAM
                for b in range(NB):
                    psum_p = ps.tile([C, 512], f32, name="psum_p", tag="pp", bufs=1)
                    for ci in range(NCH):
                        nc.tensor.matmul(
                            out=psum_p[:],
                            lhsT=projT[:, ci * C:(ci + 1) * C],
                            rhs=lch(ci)[:, b * 512:(b + 1) * 512],
                            start=(ci == 0), stop=(ci == NCH - 1))
                    pchunk = sb.tile([C, 512], f32, name="pchunk", tag="pch")
                    nc.scalar.activation(pchunk[:], psum_p[:], AF.Copy)
                    for s in range(4):
                        tps2 = ps.tile([P, C], f32, name="tps2", tag="tps", bufs=1)
                        nc.tensor.transpose(
                            out=tps2[:], in_=pchunk[:, s * P:(s + 1) * P],
                            identity=ident[0:C, 0:C])
                        ptrow = sb.tile([P, C], f32, name="ptrow", tag="ptr")
                        nc.vector.tensor_copy(out=ptrow[:], in_=tps2[:])
                        r0 = n0 + b * 512 + s * P
                        nc.sync.dma_start(out=pt_d[r0:r0 + P, :], in_=ptrow[:])

                # rsqrt of norms2 (compact) + Newton refinement
                nrow = sb.tile([1, NOCT], f32, name="nrow", tag="nrow", bufs=1)
                nc.vector.tensor_copy(out=nrow[:], in_=psum_n[0:1, :])
                ncmp = sb.tile([P, NOCT // P], f32, name="ncmp", tag="ncmp")
                nc.sync.dma_start(
                    out=ncmp[:],
                    in_=nrow[0:1, :].rearrange("a (p f) -> a p f", p=P))
                scmp = sb.tile([P, NOCT // P], f32, name="scmp", tag="scmp")
                nc.scalar.activation(scmp[:], ncmp[:], AF.Sqrt)
                r0t = sb.tile([P, NOCT // P], f32, name="r0t", tag="r0t")
                nc.vector.reciprocal(r0t[:], scmp[:])
                # Newton for rsqrt: r1 = r0*(1.5 - 0.5*n*r0^2)
                t1 = sb.tile([P, NOCT // P], f32, name="t1", tag="t1")
                nc.vector.tensor_mul(t1[:], r0t[:], r0t[:])
                nc.vector.tensor_mul(t1[:], t1[:], ncmp[:])
                nc.vector.tensor_scalar(
                    t1[:], t1[:], -0.5, 1.5,
                    op0=mybir.AluOpType.mult, op1=mybir.AluOpType.add)
                nc.vector.tensor_mul(r0t[:], r0t[:], t1[:])
                rrow = sb.tile([1, NOCT], f32, name="rrow", tag="rrow", bufs=1)
                nc.sync.dma_start(
                    out=rrow[0:1, :].rearrange("a (p f) -> a p f", p=P),
                    in_=r0t[:])
                # replicate rnorm across partitions via K=1 matmul
                for b in range(NB):
                    nc.tensor.matmul(
                        out=psum_n[:, b * 512:(b + 1) * 512],
                        lhsT=ones[0:1, :],
                        rhs=rrow[0:1, b * 512:(b + 1) * 512],
                        start=True, stop=True)

                # prescale: lut_hat = lut * rnorm (in place)
                for ci in range(NCH):
                    nc.vector.tensor_mul(lch(ci), lch(ci), psum_n[:])

                # main matmuls + per-octant top-8
                for t in range(NT):
                    gpart = gp.tile([P, NOCT], f32, name="gpart", tag="gpart")
                    for b in range(NB):
                        psum_g = ps.tile([P, 512], f32, name="psum_g",
                                         tag="pg")
                        for ci in range(NCH):
                            nc.tensor.matmul(
                                out=psum_g[:],
                                lhsT=x_all[:, ci * TSH + t * P:
                                           ci * TSH + (t + 1) * P],
                                rhs=lch(ci)[:, b * 512:(b + 1) * 512],
                                start=(ci == 0), stop=(ci == NCH - 1))
                        nc.scalar.activation(
                            gpart[:, b * 512:(b + 1) * 512], psum_g[:],
                            AF.Copy)
                    vsl = cvals[t][:, o * 8:(o + 1) * 8]
                    nc.vector.max(out=vsl, in_=gpart[:])
                    posu = sb.tile([P, 8], u32, name="posu", tag="posu")
                    nc.vector.max_index(out=posu[:], in_max=vsl,
                                        in_values=gpart[:])
                    isl = cidxf[t][:, o * 8:(o + 1) * 8]
                    nc.vector.tensor_copy(out=isl, in_=posu[:])
                    if n0:
                        nc.vector.tensor_scalar_add(isl, isl, float(n0))

            # ---- merge + gather + project ----
            for t in range(NT):
                m8 = sb.tile([P, 8], f32, name="m8", tag="m8")
                nc.vector.max(out=m8[:], in_=cvals[t][:])
                pos = sb.tile([P, 8], u32, name="pos", tag="pos")
                nc.vector.max_index(out=pos[:], in_max=m8[:],
                                    in_values=cvals[t][:])
                posf = sb.tile([P, 8], f32, name="posf", tag="posf")
                nc.vector.tensor_copy(out=posf[:], in_=pos[:])

                # one-hot extract global indices of the top-4 positions
                eq = sb.tile([P, 4 * NO * 8], f32, name="eq", tag="eq")
                iota_b = bass.AP(iota64f.tensor, iota64f[:].offset,
                                 [[iota64f[:].ap[0][0], P], [0, 4], [1, NO * 8]])
                posf_b = bass.AP(posf.tensor, posf[:].offset,
                                 [[posf[:].ap[0][0], P], [1, 4], [0, NO * 8]])
                nc.vector.tensor_tensor(out=eq[:], in0=iota_b, in1=posf_b,
                                        op=mybir.AluOpType.is_equal)
                cidx_b = bass.AP(cidxf[t].tensor, cidxf[t][:].offset,
                                 [[cidxf[t][:].ap[0][0], P], [0, 4], [1, NO * 8]])
                nc.vector.tensor_tensor(out=eq[:], in0=eq[:], in1=cidx_b,
                                        op=mybir.AluOpType.mult)
                idx4f = sb.tile([P, 4], f32, name="idx4f", tag="idx4f")
                nc.vector.tensor_reduce(
                    out=idx4f[:],
                    in_=eq[:].rearrange("p (j n) -> p j n", j=4),
                    op=mybir.AluOpType.add, axis=mybir.AxisListType.X)
                idx4u = sb.tile([P, 4], u32, name="idx4u", tag="idx4u")
                nc.vector.tensor_copy(out=idx4u[:], in_=idx4f[:])

                # gather 4 P^T rows per query, sum (mean+proj+bias prefolded)
                feats = sb.tile([P, C], f32, name="feats", tag="feats")
                gs = []
                for j in range(4):
                    g = sb.tile([P, C], f32, name=f"g{j}", tag=f"g{j}")
                    nc.gpsimd.indirect_dma_start(
                        out=g[:], out_offset=None,
                        in_=pt_d[:, :],
                        in_offset=bass.IndirectOffsetOnAxis(
                            ap=idx4u[:, j:j + 1], axis=0))
                    gs.append(g)
                nc.vector.tensor_add(feats[:], gs[0][:], gs[1][:])
                nc.vector.tensor_add(feats[:], feats[:], gs[2][:])
                nc.vector.tensor_add(feats[:], feats[:], gs[3][:])

                # transpose [P, C] -> [C, P] and store
                tfs = ps.tile([C, P], f32, name="tfs", tag="tps", bufs=1)
                nc.tensor.transpose(out=tfs[:], in_=feats[:],
                                    identity=ident[:])
                osb = sb.tile([C, P], f32, name="osb", tag="osb")
                nc.vector.tensor_scalar(osb[:], tfs[:], pb_sb[:, 0:1], None,
                                        op0=mybir.AluOpType.add)
                nc.sync.dma_start(out=out_d[:, t * P:(t + 1) * P], in_=osb[:])

    nc.compile()
    return nc


_NC_CACHE = None
LAST_EXEC_NS = None


def kernel(x, lut, proj_w, proj_b, k):
    global _NC_CACHE, LAST_EXEC_NS
    assert int(k) == K
    x = np.asarray(x, dtype=np.float32)
    lut_f = np.ascontiguousarray(np.asarray(lut, dtype=np.float32)[0])
    pw = np.ascontiguousarray(np.asarray(proj_w, dtype=np.float32))
    pb = np.asarray(proj_b, dtype=np.float32).reshape(C, 1)

    if _NC_CACHE is None:
        _NC_CACHE = build_kernel()
    nc = _NC_CACHE

    in_maps = []
    for core in range(NCORES):
        xs = np.ascontiguousarray(x[0][:, core * TSH:(core + 1) * TSH])
        in_maps.append({"xs": xs, "lut": lut_f, "projw": pw, "projb": pb})

    import os
    trace = bool(int(os.environ.get("KERNEL_TRACE", "0")))
    res = bass_utils.run_bass_kernel_spmd(nc, in_maps,
                                          core_ids=list(range(NCORES)),
                                          trace=trace)
    LAST_EXEC_NS = res.exec_time_ns
    out = np.empty((B, C, T), dtype=np.float32)
    for core in range(NCORES):
        out[0][:, core * TSH:(core + 1) * TSH] = res.results[core]["out"]
    return out

